# revision 25
# baseline (speedup 1.0000x reference)
"""Trainium2 Bass kernel for the LRU (Linear Recurrent Unit) nn.Module.

Math
----
Reference computes, per timestep t (T=4096, H=2048, N=1024):
    Bu_t   = B_norm @ u_t                    (complex, B_norm = (B_re+iB_im)*gamma)
    h_t    = lambda * h_{t-1} + Bu_t         (diagonal complex recurrence)
    y_t    = Re(C @ h_t) + D * u_t

Device strategy (8 NeuronCores, tensor-parallel over d_hidden N):
Each core owns NSH = N/8 = 128 channels.  With lambda_n = r_n * exp(i*theta_n)
the rotating-frame substitution g_t = exp(-i*theta*t) * h_t turns the complex
recurrence into two *real* scans
    g_t = r * g_{t-1} + exp(-i*theta*t) * Bu_t
which map 1:1 onto the VectorE tensor_tensor_scan instruction.  Rotation
tables cos(theta_n*t), sin(theta_n*t) are precomputed on host in float64.

Per core (all matmul operands bf16, accumulation in f32, scan state f32):
  mm1  (TensorE):  Bu.T = BnT.T @ x.T        -> [NSH, T] (re,im) in PSUM
  cast (ScalarE):  bure/buim = bf16(PSUM)    -> SBUF  (frees PSUM early and
                   lets every following DVE op run in 2x 16-bit mode)
  rot-in (VectorE, bf16 2x): w = exp(-i theta t) Bu
  scan (VectorE):  g = scan(r, w)            (f32 coeff/state, bf16 out)
  rot-out (VectorE, bf16 2x): h = exp(+i theta t) g
  mm2  (TensorE):  y_part = h_re.T @ C_re.T - h_im.T @ C_im.T  -> [T, H]
  drain: mm2 PSUM tiles alternate between ScalarE and VectorE copies so the
         drain sustains the mm2 matmul issue rate (one engine per tile).
Host gathers: y = sum_c y_part_c + D * u  (float64).

TensorE is the roofline engine (512 matmuls x 512 moving columns, ~216ns
issue-to-issue warm).  The schedule keeps it saturated:
 - chunk-0 x and the mm1 weights are loaded as interleaved per-k-tile groups
   over both HWDGE queues so mm1(0) chases the DMA down the k index; the
   C matrix (needed ~25us in) is deferred behind them,
 - warm-up matmuls run during the initial DMA so the PE HAM clock-gate
   releases before real work,
 - mm2 of chunk c is emitted after mm1 of chunk c+1,
 - the last chunk's vector chain runs in halves, with mm2(7) time-tiles
   interleaved after each half, and back(6) is emitted before front(7) so
   its drain copies are not queued behind chunk-7 vector work.
"""

import os

import numpy as np

T, H, N = 4096, 2048, 1024
NCORES = 8
NSH = N // NCORES  # 128 channels per core
TCH = 512          # time chunk (= max matmul moving free dim = 1 PSUM bank)
NCHUNK = T // TCH  # 8
KT = H // 128      # 16 contraction tiles in mm1
HCH = 512          # h chunk in mm2
NHC = H // HCH     # 4
NWARM = 36         # HAM warm-up matmuls (F=128) before the first real mm1

_CACHE = {}

# last BassKernelResults (for test harness introspection)
last_results = None


def _build_program():
    import concourse.mybir as mybir
    from concourse import bacc
    from concourse.tile import TileContext

    F32 = mybir.dt.float32
    BF16 = mybir.dt.bfloat16
    FP8 = mybir.dt.float8e3
    MUL = mybir.AluOpType.mult
    ADD = mybir.AluOpType.add
    SUB = mybir.AluOpType.subtract

    nc = bacc.Bacc("TRN2", target_bir_lowering=False, debug=False,
                   num_devices=NCORES)

    xT = nc.dram_tensor("xT", [128, NCHUNK * KT * TCH], FP8,
                        kind="ExternalInput").ap()
    bn_re = nc.dram_tensor("bn_re", [128, KT * NSH], BF16,
                           kind="ExternalInput").ap()
    bn_im = nc.dram_tensor("bn_im", [128, KT * NSH], BF16,
                           kind="ExternalInput").ap()
    ct_re = nc.dram_tensor("ct_re", [NSH, H], BF16, kind="ExternalInput").ap()
    ct_in = nc.dram_tensor("ct_in", [NSH, H], BF16, kind="ExternalInput").ap()
    # merged per-chunk rotation table: [128, c, (cos|sin), TCH]
    csT = nc.dram_tensor("csT", [NSH, NCHUNK * 2 * TCH], BF16,
                         kind="ExternalInput").ap()
    rvec = nc.dram_tensor("rvec", [NSH, 1], F32, kind="ExternalInput").ap()
    ypart = nc.dram_tensor("ypart", [T, H], BF16, kind="ExternalOutput").ap()

    with TileContext(nc) as tc:
        with (
            tc.tile_pool(name="persist", bufs=1) as pp,
            tc.tile_pool(name="xin", bufs=6) as xp,
            tc.tile_pool(name="bu", bufs=2) as bp,
            tc.tile_pool(name="rot", bufs=2) as rp,
            tc.tile_pool(name="wbuf", bufs=2) as wp,
            tc.tile_pool(name="gbuf", bufs=2) as gp,
            tc.tile_pool(name="hbuf", bufs=2) as hp,
            tc.tile_pool(name="yout", bufs=3) as yp,
            tc.tile_pool(name="csn", bufs=3) as cp,
            tc.tile_pool(name="ps1", bufs=1, space="PSUM") as ps1,
            tc.tile_pool(name="ps2", bufs=6, space="PSUM") as ps2,
        ):
            # ---- startup loads, in consumption order -------------------
            # gpsimd (SWDGE) gets the small early pieces + the deferred C.
            rv = pp.tile([128, 1], F32, tag="rv")
            nc.gpsimd.dma_start(rv[:], rvec)
            cs0 = cp.tile([128, 2 * TCH], BF16, tag="cs")
            nc.gpsimd.dma_start(cs0[:], csT[:, 0:2 * TCH])

            # chunk-0 x + mm1 weights ride both HWDGE queues as interleaved
            # per-(2 k-tile) groups so mm1(0) can chase the DMA down the
            # k-tile index: group k = {bre,bim cols [2k..2k+2), x k-tiles
            # 2k, 2k+1}, split sync/scalar.
            bre = pp.tile([128, KT * NSH], BF16, tag="bre")
            bim = pp.tile([128, KT * NSH], BF16, tag="bim")
            xt0 = xp.tile([128, KT * TCH], FP8, tag="xt")
            for k in range(KT // 2):
                b0, b1 = 2 * k * NSH, (2 * k + 2) * NSH
                nc.sync.dma_start(bre[:, b0:b1], bn_re[:, b0:b1])
                nc.scalar.dma_start(bim[:, b0:b1], bn_im[:, b0:b1])
                xa, xb = 2 * k * TCH, (2 * k + 1) * TCH
                nc.sync.dma_start(xt0[:, xa:xa + TCH], xT[:, xa:xa + TCH])
                nc.scalar.dma_start(xt0[:, xb:xb + TCH], xT[:, xb:xb + TCH])
            rbc = pp.tile([128, TCH], F32, tag="rbc")
            nc.vector.tensor_copy(rbc[:], rv[:, 0:1].broadcast_to([128, TCH]))

            # chunk-1/2 x ride sync + gpsimd (ahead of the deferred C)
            xt1 = xp.tile([128, KT * TCH], FP8, tag="xt")
            QW1 = KT * TCH // 4
            for q in range(4):
                eng = nc.sync if q % 2 == 0 else nc.gpsimd
                eng.dma_start(xt1[:, q * QW1:(q + 1) * QW1],
                              xT[:, KT * TCH + q * QW1:KT * TCH + (q + 1) * QW1])
            xt2 = xp.tile([128, KT * TCH], FP8, tag="xt")
            for q in range(4):
                eng = nc.sync if q % 2 == 0 else nc.gpsimd
                x2o = 2 * KT * TCH
                eng.dma_start(xt2[:, q * QW1:(q + 1) * QW1],
                              xT[:, x2o + q * QW1:x2o + (q + 1) * QW1])

            # deferred: C (first needed by mm2(0), ~25us in)
            ctr = pp.tile([128, H], BF16, tag="ctr")
            nc.gpsimd.dma_start(ctr[:], ct_re)
            cti = pp.tile([128, H], BF16, tag="cti")
            nc.gpsimd.dma_start(cti[:], ct_in)

            # ---- HAM warm-up: keep the PE busy while DMAs fill SBUF so the
            # clock-gate releases (1.2->2.4 GHz) before the first real mm1.
            wj = pp.tile([128, 128], BF16, tag="wj")
            nc.vector.memset(wj[:], 0)
            wmp = ps1.tile([128, TCH], F32, tag="pre")
            for _ in range(NWARM):
                nc.tensor.matmul(wmp[:, 0:128], wj[:], wj[:],
                                 start=True, stop=True)

            prev_gre = prev_gim = None
            hist = []  # pending (chunk, hre, him) awaiting mm2

            def emit_front(c, xt_pre=None, halves=None, post_half=None):
                """mm1 + cast + rotations + scans for chunk c.

                halves: optional list of time-slices for the vector chain
                (default: one full-chunk slice).  post_half(hi) is called
                after each slice's vector ops are emitted (used to
                interleave the last chunk's mm2).
                """
                nonlocal prev_gre, prev_gim
                if xt_pre is not None:
                    xt = xt_pre
                else:
                    # later chunks: 4 quarters split across both HWDGE queues
                    xt = xp.tile([128, KT * TCH], FP8, tag="xt")
                    x0 = c * KT * TCH
                    QW = KT * TCH // 4
                    for q in range(4):
                        # all mid-kernel loads ride the sync queue: a DMA
                        # issue blocked on ring pacing must never sit ahead
                        # of ACT casts on the scalar queue (HOL blocking)
                        nc.sync.dma_start(
                            xt[:, q * QW:(q + 1) * QW],
                            xT[:, x0 + q * QW:x0 + (q + 1) * QW],
                        )
                pre = ps1.tile([128, TCH], F32, tag="pre")
                pim = ps1.tile([128, TCH], F32, tag="pim")
                for a in range(KT):
                    xsl = xt[:, a * TCH:(a + 1) * TCH]
                    nc.tensor.matmul(
                        pre[:], bre[:, a * NSH:(a + 1) * NSH], xsl,
                        start=(a == 0), stop=(a == KT - 1),
                    )
                    nc.tensor.matmul(
                        pim[:], bim[:, a * NSH:(a + 1) * NSH], xsl,
                        start=(a == 0), stop=(a == KT - 1),
                    )
                # merged cos|sin table for this chunk, one DMA
                if c == 0:
                    cs_t = cs0
                else:
                    cs_t = cp.tile([128, 2 * TCH], BF16, tag="cs")
                    nc.gpsimd.dma_start(
                        cs_t[:], csT[:, c * 2 * TCH:(c + 1) * 2 * TCH])
                csl = cs_t[:, 0:TCH]
                snl = cs_t[:, TCH:2 * TCH]
                # PSUM -> bf16 SBUF on ScalarE: frees mm1's PSUM banks and
                # puts all rotation operands in 16-bit SBUF (2x DVE rate).
                bure = bp.tile([128, TCH], BF16, tag="bure")
                buim = bp.tile([128, TCH], BF16, tag="buim")
                nc.scalar.copy(bure[:], pre[:])
                nc.scalar.copy(buim[:], pim[:])
                t1 = rp.tile([128, TCH], BF16, tag="t1")
                t2 = rp.tile([128, TCH], BF16, tag="t2")
                wre = wp.tile([128, TCH], BF16, tag="wre")
                wim = wp.tile([128, TCH], BF16, tag="wim")
                gre = gp.tile([128, TCH], BF16, tag="gre")
                gim = gp.tile([128, TCH], BF16, tag="gim")
                hre = hp.tile([128, TCH], BF16, tag="hre")
                him = hp.tile([128, TCH], BF16, tag="him")
                hist.append((c, hre, him))
                if halves is None:
                    halves = [slice(0, TCH)]
                for hi, hs in enumerate(halves):
                    cs_h, sn_h = csl[:, hs], snl[:, hs]
                    t1_h, t2_h = t1[:, hs], t2[:, hs]
                    # rotate into the r-frame: w = e^{-i theta t} * Bu
                    nc.vector.tensor_tensor(t1_h, cs_h, bure[:, hs], op=MUL)
                    nc.vector.tensor_tensor(t2_h, sn_h, buim[:, hs], op=MUL)
                    nc.vector.tensor_tensor(wre[:, hs], t1_h, t2_h, op=ADD)
                    nc.vector.tensor_tensor(t1_h, cs_h, buim[:, hs], op=MUL)
                    nc.vector.tensor_tensor(t2_h, sn_h, bure[:, hs], op=MUL)
                    nc.vector.tensor_tensor(wim[:, hs], t1_h, t2_h, op=SUB)
                    # the two real scans (f32 coeff + internal state)
                    if hi == 0:
                        init_re = 0.0 if c == 0 else prev_gre[:, TCH - 1:TCH]
                        init_im = 0.0 if c == 0 else prev_gim[:, TCH - 1:TCH]
                    else:
                        init_re = gre[:, hs.start - 1:hs.start]
                        init_im = gim[:, hs.start - 1:hs.start]
                    nc.vector.tensor_tensor_scan(
                        gre[:, hs], rbc[:, hs], wre[:, hs], init_re, MUL, ADD)
                    nc.vector.tensor_tensor_scan(
                        gim[:, hs], rbc[:, hs], wim[:, hs], init_im, MUL, ADD)
                    # rotate back: h = e^{+i theta t} * g
                    gre_h, gim_h = gre[:, hs], gim[:, hs]
                    nc.vector.tensor_tensor(t1_h, cs_h, gre_h, op=MUL)
                    nc.vector.tensor_tensor(t2_h, sn_h, gim_h, op=MUL)
                    nc.vector.tensor_tensor(hre[:, hs], t1_h, t2_h, op=SUB)
                    nc.vector.tensor_tensor(t1_h, cs_h, gim_h, op=MUL)
                    nc.vector.tensor_tensor(t2_h, sn_h, gre_h, op=MUL)
                    nc.vector.tensor_tensor(him[:, hs], t1_h, t2_h, op=ADD)
                    if post_half is not None:
                        post_half(hi)
                prev_gre, prev_gim = gre, gim

            def emit_back_tt(c, hre, him, tt):
                """One 128-timestep tile of mm2 + its drain + store."""
                t0 = c * TCH
                last_c = c == NCHUNK - 1
                lre = hre[:, tt * 128:(tt + 1) * 128]
                lim = him[:, tt * 128:(tt + 1) * 128]
                yo = yp.tile([128, H], BF16, tag="yo")
                pos = []
                for _ in range(NHC):
                    po = ps2.tile([128, HCH], F32, tag="po")
                    pos.append(po)
                for hc in range(NHC):
                    nc.tensor.matmul(
                        pos[hc][:], lre, ctr[:, hc * HCH:(hc + 1) * HCH],
                        start=True, stop=False,
                    )
                for hc in range(NHC):
                    nc.tensor.matmul(
                        pos[hc][:], lim, cti[:, hc * HCH:(hc + 1) * HCH],
                        start=False, stop=True,
                    )
                SSP = 288
                for hc in range(NHC):
                    ysl = yo[:, hc * HCH:(hc + 1) * HCH]
                    nc.scalar.copy(ysl[:, 0:SSP], pos[hc][:, 0:SSP])
                    nc.vector.tensor_copy(ysl[:, SSP:HCH],
                                          pos[hc][:, SSP:HCH])
                r0 = t0 + tt * 128
                if last_c:
                    # finest store pieces over both HWDGE queues: short tail
                    nc.sync.dma_start(ypart[r0:r0 + 32, :], yo[0:32, :])
                    nc.scalar.dma_start(ypart[r0 + 32:r0 + 64, :],
                                        yo[32:64, :])
                    nc.sync.dma_start(ypart[r0 + 64:r0 + 96, :], yo[64:96, :])
                    nc.scalar.dma_start(ypart[r0 + 96:r0 + 128, :],
                                        yo[96:128, :])
                else:
                    e0 = nc.gpsimd
                    e1 = nc.sync if tt % 2 == 0 else nc.gpsimd
                    e0.dma_start(ypart[r0:r0 + 64, :], yo[0:64, :])
                    e1.dma_start(ypart[r0 + 64:r0 + 128, :], yo[64:128, :])

            def emit_back():
                """mm2 + output for the oldest pending chunk."""
                c, hre, him = hist.pop(0)
                for tt in range(TCH // 128):
                    emit_back_tt(c, hre, him, tt)

            def emit_last_half(c, ctx, hi):
                """One F=256 half-time piece of the last chunk: mm1 half +
                cast + vector chain.  Half 0 is emitted before back(6) so
                its vector work overlaps mm2(6) on the other engines."""
                nonlocal prev_gre, prev_gim
                HT = TCH // 2
                hs = slice(0, HT) if hi == 0 else slice(HT, TCH)
                xt, cs_t, pre, pim = (ctx["xt"], ctx["cs"], ctx["pre"],
                                      ctx["pim"])
                for a in range(KT):
                    xsl = xt[:, a * TCH + hs.start:a * TCH + hs.stop]
                    nc.tensor.matmul(
                        pre[:, hs], bre[:, a * NSH:(a + 1) * NSH], xsl,
                        start=(a == 0), stop=(a == KT - 1),
                    )
                    nc.tensor.matmul(
                        pim[:, hs], bim[:, a * NSH:(a + 1) * NSH], xsl,
                        start=(a == 0), stop=(a == KT - 1),
                    )
                csl = cs_t[:, 0:TCH]
                snl = cs_t[:, TCH:2 * TCH]
                bure, buim = ctx["bure"], ctx["buim"]
                t1, t2 = ctx["t1"], ctx["t2"]
                wre, wim = ctx["wre"], ctx["wim"]
                gre, gim = ctx["gre"], ctx["gim"]
                hre, him = ctx["hre"], ctx["him"]
                nc.scalar.copy(bure[:, hs], pre[:, hs])
                nc.scalar.copy(buim[:, hs], pim[:, hs])
                cs_h, sn_h = csl[:, hs], snl[:, hs]
                t1_h, t2_h = t1[:, hs], t2[:, hs]
                nc.vector.tensor_tensor(t1_h, cs_h, bure[:, hs], op=MUL)
                nc.vector.tensor_tensor(t2_h, sn_h, buim[:, hs], op=MUL)
                nc.vector.tensor_tensor(wre[:, hs], t1_h, t2_h, op=ADD)
                nc.vector.tensor_tensor(t1_h, cs_h, buim[:, hs], op=MUL)
                nc.vector.tensor_tensor(t2_h, sn_h, bure[:, hs], op=MUL)
                nc.vector.tensor_tensor(wim[:, hs], t1_h, t2_h, op=SUB)
                if hi == 0:
                    init_re = prev_gre[:, TCH - 1:TCH]
                    init_im = prev_gim[:, TCH - 1:TCH]
                else:
                    init_re = gre[:, hs.start - 1:hs.start]
                    init_im = gim[:, hs.start - 1:hs.start]
                nc.vector.tensor_tensor_scan(
                    gre[:, hs], rbc[:, hs], wre[:, hs], init_re, MUL, ADD)
                nc.vector.tensor_tensor_scan(
                    gim[:, hs], rbc[:, hs], wim[:, hs], init_im, MUL, ADD)
                gre_h, gim_h = gre[:, hs], gim[:, hs]
                nc.vector.tensor_tensor(t1_h, cs_h, gre_h, op=MUL)
                nc.vector.tensor_tensor(t2_h, sn_h, gim_h, op=MUL)
                nc.vector.tensor_tensor(hre[:, hs], t1_h, t2_h, op=SUB)
                nc.vector.tensor_tensor(t1_h, cs_h, gim_h, op=MUL)
                nc.vector.tensor_tensor(t2_h, sn_h, gre_h, op=MUL)
                nc.vector.tensor_tensor(him[:, hs], t1_h, t2_h, op=ADD)
                if hi == 1:
                    prev_gre, prev_gim = gre, gim

            LAST = NCHUNK - 1
            XPRE = {0: xt0, 1: xt1, 2: xt2}
            for c in range(LAST):
                emit_front(c, xt_pre=XPRE.get(c))
                if c >= 1:
                    emit_back()
            # Last chunk: half 0 (mm1 F=256 + vector chain) is emitted
            # before back(6), so TensorE runs mm2(6) while VectorE rotates
            # half 0; then half 1, then mm2(7) tiles chase the halves.
            xtL = xp.tile([128, KT * TCH], FP8, tag="xt")
            xL0 = LAST * KT * TCH
            QW = KT * TCH // 4
            for q in range(4):
                nc.sync.dma_start(xtL[:, q * QW:(q + 1) * QW],
                                  xT[:, xL0 + q * QW:xL0 + (q + 1) * QW])
            csL = cp.tile([128, 2 * TCH], BF16, tag="cs")
            nc.gpsimd.dma_start(
                csL[:], csT[:, LAST * 2 * TCH:(LAST + 1) * 2 * TCH])
            ctx = {"xt": xtL, "cs": csL}
            for nm, pool, dt in (
                ("pre", ps1, F32), ("pim", ps1, F32),
                ("bure", bp, BF16), ("buim", bp, BF16),
                ("t1", rp, BF16), ("t2", rp, BF16),
                ("wre", wp, BF16), ("wim", wp, BF16),
                ("gre", gp, BF16), ("gim", gp, BF16),
                ("hre", hp, BF16), ("him", hp, BF16),
            ):
                ctx[nm] = pool.tile([128, TCH], dt, tag=nm, name=f"L_{nm}")
            emit_last_half(LAST, ctx, 0)
            emit_back()  # chunk 6
            emit_last_half(LAST, ctx, 1)
            for tt in range(TCH // 128):
                emit_back_tt(LAST, ctx["hre"], ctx["him"], tt)

    nc.compile()
    return nc


def _arrange_bn(bn_slice):
    import ml_dtypes
    # bn_slice [NSH, H] (float64) -> [128, KT*NSH] with
    # out[p, a*NSH + n] = bn_slice[n, a*128 + p]
    bnT = bn_slice.T.astype(ml_dtypes.bfloat16)  # [H, NSH]
    return np.ascontiguousarray(
        bnT.reshape(KT, 128, NSH).transpose(1, 0, 2)).reshape(128, -1)


def _host_prep(inputs, nu, theta, gamma_log, B_re, B_im, C_re, C_im, D):
    """Float64 host-side precompute; returns per-core input maps."""
    import ml_dtypes
    BF = ml_dtypes.bfloat16
    x = np.asarray(inputs, dtype=np.float32)
    th64 = np.exp(np.asarray(theta).astype(np.float64))
    r64 = np.exp(-np.exp(np.asarray(nu).astype(np.float64)))
    gamma = np.exp(np.asarray(gamma_log).astype(np.float64))
    Bn_re = np.asarray(B_re).astype(np.float64) * gamma[:, None]
    Bn_im = np.asarray(B_im).astype(np.float64) * gamma[:, None]
    t_idx = np.arange(T, dtype=np.float64)
    phase = th64[:, None] * t_idx[None, :]
    cos_all = np.cos(phase).astype(BF)  # [N, T]
    sin_all = np.sin(phase).astype(BF)
    # merged per-chunk layout: cs_all[n, c, 0|1, t] = cos|sin(th_n*(c*TCH+t))
    cs_all = np.stack(
        [cos_all.reshape(N, NCHUNK, TCH), sin_all.reshape(N, NCHUNK, TCH)],
        axis=2).reshape(N, NCHUNK * 2 * TCH)
    # pre-arrange x into the per-chunk SBUF layout:
    # xTa[p, c, a, t] = x[c*TCH + t, a*128 + p]
    E3 = ml_dtypes.float8_e3m4
    xTa = np.ascontiguousarray(
        x.reshape(NCHUNK, TCH, KT, 128).transpose(3, 0, 2, 1).astype(E3)
    ).reshape(128, -1)
    C_re = np.asarray(C_re, dtype=np.float32).astype(BF)
    C_im = np.asarray(C_im, dtype=np.float32).astype(BF)

    in_maps = []
    for c in range(NCORES):
        sl = slice(c * NSH, (c + 1) * NSH)
        in_maps.append({
            "xT": xTa,
            "bn_re": _arrange_bn(Bn_re[sl]),
            "bn_im": _arrange_bn(Bn_im[sl]),
            "ct_re": np.ascontiguousarray(C_re[:, sl].T),
            "ct_in": np.ascontiguousarray(-C_im[:, sl].T),
            "csT": np.ascontiguousarray(cs_all[sl]),
            "rvec": np.ascontiguousarray(r64[sl].astype(np.float32)[:, None]),
        })
    return in_maps


def kernel(inputs, nu, theta, gamma_log, B_re, B_im, C_re, C_im, D):
    global last_results
    from concourse.bass_utils import run_bass_kernel_spmd

    if "nc" not in _CACHE:
        _CACHE["nc"] = _build_program()
    nc = _CACHE["nc"]

    in_maps = _host_prep(
        inputs, nu, theta, gamma_log, B_re, B_im, C_re, C_im, D)

    trace = os.environ.get("LRU_TRACE") == "1"
    res = run_bass_kernel_spmd(
        nc, in_maps, core_ids=list(range(NCORES)), trace=trace)
    last_results = res

    y64 = np.zeros((T, H), np.float64)
    for r in res.results:
        y64 += r["ypart"].astype(np.float64)
    y64 += (np.asarray(D).astype(np.float64)[None, :]
            * np.asarray(inputs).astype(np.float64))
    return y64.astype(np.float32)


# revision 26
# speedup vs baseline: 1.0226x; 1.0226x over previous
"""Trainium2 Bass kernel for the LRU (Linear Recurrent Unit) nn.Module.

Math
----
Reference computes, per timestep t (T=4096, H=2048, N=1024):
    Bu_t   = B_norm @ u_t                    (complex, B_norm = (B_re+iB_im)*gamma)
    h_t    = lambda * h_{t-1} + Bu_t         (diagonal complex recurrence)
    y_t    = Re(C @ h_t) + D * u_t

Device strategy (8 NeuronCores, tensor-parallel over d_hidden N):
Each core owns NSH = N/8 = 128 channels.  With lambda_n = r_n * exp(i*theta_n)
the rotating-frame substitution g_t = exp(-i*theta*t) * h_t turns the complex
recurrence into two *real* scans
    g_t = r * g_{t-1} + exp(-i*theta*t) * Bu_t
which map 1:1 onto the VectorE tensor_tensor_scan instruction.  Rotation
tables cos(theta_n*t), sin(theta_n*t) are precomputed on host in float64.

Per core (all matmul operands bf16, accumulation in f32, scan state f32):
  mm1  (TensorE):  Bu.T = BnT.T @ x.T        -> [NSH, T] (re,im) in PSUM
  cast (ScalarE):  bure/buim = bf16(PSUM)    -> SBUF  (frees PSUM early and
                   lets every following DVE op run in 2x 16-bit mode)
  rot-in (VectorE, bf16 2x): w = exp(-i theta t) Bu
  scan (VectorE):  g = scan(r, w)            (f32 coeff/state, bf16 out)
  rot-out (VectorE, bf16 2x): h = exp(+i theta t) g
  mm2  (TensorE):  y_part = h_re.T @ C_re.T - h_im.T @ C_im.T  -> [T, H]
  drain: mm2 PSUM tiles alternate between ScalarE and VectorE copies so the
         drain sustains the mm2 matmul issue rate (one engine per tile).
Host gathers: y = sum_c y_part_c + D * u  (float64).

TensorE is the roofline engine (512 matmuls x 512 moving columns, ~216ns
issue-to-issue warm).  The schedule keeps it saturated:
 - chunk-0 x and the mm1 weights are loaded as interleaved per-k-tile groups
   over both HWDGE queues so mm1(0) chases the DMA down the k index; the
   C matrix (needed ~25us in) is deferred behind them,
 - warm-up matmuls run during the initial DMA so the PE HAM clock-gate
   releases before real work,
 - mm2 of chunk c is emitted after mm1 of chunk c+1,
 - the last chunk's vector chain runs in halves, with mm2(7) time-tiles
   interleaved after each half, and back(6) is emitted before front(7) so
   its drain copies are not queued behind chunk-7 vector work.
"""

import os

import numpy as np

T, H, N = 4096, 2048, 1024
NCORES = 8
NSH = N // NCORES  # 128 channels per core
TCH = 512          # time chunk (= max matmul moving free dim = 1 PSUM bank)
NCHUNK = T // TCH  # 8
KT = H // 128      # 16 contraction tiles in mm1
HCH = 512          # h chunk in mm2
NHC = H // HCH     # 4
NWARM = 36         # HAM warm-up matmuls (F=128) before the first real mm1

_CACHE = {}

# last BassKernelResults (for test harness introspection)
last_results = None


def _build_program():
    import concourse.mybir as mybir
    from concourse import bacc
    from concourse.tile import TileContext

    F32 = mybir.dt.float32
    BF16 = mybir.dt.bfloat16
    FP8 = mybir.dt.float8e3
    MUL = mybir.AluOpType.mult
    ADD = mybir.AluOpType.add
    SUB = mybir.AluOpType.subtract

    nc = bacc.Bacc("TRN2", target_bir_lowering=False, debug=False,
                   num_devices=NCORES)

    xT = nc.dram_tensor("xT", [128, NCHUNK * KT * TCH], FP8,
                        kind="ExternalInput").ap()
    bn_re = nc.dram_tensor("bn_re", [128, KT * NSH], BF16,
                           kind="ExternalInput").ap()
    bn_im = nc.dram_tensor("bn_im", [128, KT * NSH], BF16,
                           kind="ExternalInput").ap()
    ct_re = nc.dram_tensor("ct_re", [NSH, H], BF16, kind="ExternalInput").ap()
    ct_in = nc.dram_tensor("ct_in", [NSH, H], BF16, kind="ExternalInput").ap()
    # merged per-chunk rotation table: [128, c, (cos|sin), TCH]
    csT = nc.dram_tensor("csT", [NSH, NCHUNK * 2 * TCH], BF16,
                         kind="ExternalInput").ap()
    rvec = nc.dram_tensor("rvec", [NSH, 1], F32, kind="ExternalInput").ap()
    ypart = nc.dram_tensor("ypart", [T, H], BF16, kind="ExternalOutput").ap()

    with TileContext(nc) as tc:
        with (
            tc.tile_pool(name="persist", bufs=1) as pp,
            tc.tile_pool(name="xin", bufs=6) as xp,
            tc.tile_pool(name="bu", bufs=2) as bp,
            tc.tile_pool(name="rot", bufs=2) as rp,
            tc.tile_pool(name="wbuf", bufs=2) as wp,
            tc.tile_pool(name="gbuf", bufs=2) as gp,
            tc.tile_pool(name="hbuf", bufs=2) as hp,
            tc.tile_pool(name="yout", bufs=3) as yp,
            tc.tile_pool(name="csn", bufs=3) as cp,
            tc.tile_pool(name="ps1", bufs=1, space="PSUM") as ps1,
            tc.tile_pool(name="ps2", bufs=6, space="PSUM") as ps2,
        ):
            # ---- startup loads, in consumption order -------------------
            # gpsimd (SWDGE) gets the small early pieces + the deferred C.
            rv = pp.tile([128, 1], F32, tag="rv")
            nc.gpsimd.dma_start(rv[:], rvec)
            cs0 = cp.tile([128, 2 * TCH], BF16, tag="cs")
            nc.gpsimd.dma_start(cs0[:], csT[:, 0:2 * TCH])

            # chunk-0 x + mm1 weights ride both HWDGE queues as interleaved
            # per-(2 k-tile) groups so mm1(0) can chase the DMA down the
            # k-tile index: group k = {bre,bim cols [2k..2k+2), x k-tiles
            # 2k, 2k+1}, split sync/scalar.
            bre = pp.tile([128, KT * NSH], BF16, tag="bre")
            bim = pp.tile([128, KT * NSH], BF16, tag="bim")
            xt0 = xp.tile([128, KT * TCH], FP8, tag="xt")
            for k in range(KT // 2):
                b0, b1 = 2 * k * NSH, (2 * k + 2) * NSH
                nc.sync.dma_start(bre[:, b0:b1], bn_re[:, b0:b1])
                nc.scalar.dma_start(bim[:, b0:b1], bn_im[:, b0:b1])
                xa, xb = 2 * k * TCH, (2 * k + 1) * TCH
                nc.sync.dma_start(xt0[:, xa:xa + TCH], xT[:, xa:xa + TCH])
                nc.scalar.dma_start(xt0[:, xb:xb + TCH], xT[:, xb:xb + TCH])
            rbc = pp.tile([128, TCH], F32, tag="rbc")
            nc.vector.tensor_copy(rbc[:], rv[:, 0:1].broadcast_to([128, TCH]))

            # chunk-1/2 x ride sync + gpsimd (ahead of the deferred C)
            xt1 = xp.tile([128, KT * TCH], FP8, tag="xt")
            QW1 = KT * TCH // 4
            for q in range(4):
                eng = nc.sync if q % 2 == 0 else nc.gpsimd
                eng.dma_start(xt1[:, q * QW1:(q + 1) * QW1],
                              xT[:, KT * TCH + q * QW1:KT * TCH + (q + 1) * QW1])
            xt2 = xp.tile([128, KT * TCH], FP8, tag="xt")
            for q in range(4):
                eng = nc.sync if q % 2 == 0 else nc.gpsimd
                x2o = 2 * KT * TCH
                eng.dma_start(xt2[:, q * QW1:(q + 1) * QW1],
                              xT[:, x2o + q * QW1:x2o + (q + 1) * QW1])

            # deferred: C (first needed by mm2(0), ~25us in)
            ctr = pp.tile([128, H], BF16, tag="ctr")
            nc.gpsimd.dma_start(ctr[:], ct_re)
            cti = pp.tile([128, H], BF16, tag="cti")
            nc.gpsimd.dma_start(cti[:], ct_in)

            # ---- HAM warm-up: keep the PE busy while DMAs fill SBUF so the
            # clock-gate releases (1.2->2.4 GHz) before the first real mm1.
            wj = pp.tile([128, 128], BF16, tag="wj")
            nc.vector.memset(wj[:], 0)
            wmp = ps1.tile([128, TCH], F32, tag="pre")
            for _ in range(NWARM):
                nc.tensor.matmul(wmp[:, 0:128], wj[:], wj[:],
                                 start=True, stop=True)

            prev_gre = prev_gim = None
            hist = []  # pending (chunk, hre, him) awaiting mm2

            def emit_front(c, xt_pre=None, halves=None, post_half=None):
                """mm1 + cast + rotations + scans for chunk c.

                halves: optional list of time-slices for the vector chain
                (default: one full-chunk slice).  post_half(hi) is called
                after each slice's vector ops are emitted (used to
                interleave the last chunk's mm2).
                """
                nonlocal prev_gre, prev_gim
                if xt_pre is not None:
                    xt = xt_pre
                else:
                    # later chunks: 4 quarters split across both HWDGE queues
                    xt = xp.tile([128, KT * TCH], FP8, tag="xt")
                    x0 = c * KT * TCH
                    QW = KT * TCH // 4
                    for q in range(4):
                        # all mid-kernel loads ride the sync queue: a DMA
                        # issue blocked on ring pacing must never sit ahead
                        # of ACT casts on the scalar queue (HOL blocking)
                        nc.sync.dma_start(
                            xt[:, q * QW:(q + 1) * QW],
                            xT[:, x0 + q * QW:x0 + (q + 1) * QW],
                        )
                pre = ps1.tile([128, TCH], F32, tag="pre")
                pim = ps1.tile([128, TCH], F32, tag="pim")
                for a in range(KT):
                    xsl = xt[:, a * TCH:(a + 1) * TCH]
                    nc.tensor.matmul(
                        pre[:], bre[:, a * NSH:(a + 1) * NSH], xsl,
                        start=(a == 0), stop=(a == KT - 1),
                    )
                    nc.tensor.matmul(
                        pim[:], bim[:, a * NSH:(a + 1) * NSH], xsl,
                        start=(a == 0), stop=(a == KT - 1),
                    )
                # merged cos|sin table for this chunk, one DMA
                if c == 0:
                    cs_t = cs0
                else:
                    cs_t = cp.tile([128, 2 * TCH], BF16, tag="cs")
                    nc.gpsimd.dma_start(
                        cs_t[:], csT[:, c * 2 * TCH:(c + 1) * 2 * TCH])
                csl = cs_t[:, 0:TCH]
                snl = cs_t[:, TCH:2 * TCH]
                # PSUM -> bf16 SBUF on ScalarE: frees mm1's PSUM banks and
                # puts all rotation operands in 16-bit SBUF (2x DVE rate).
                bure = bp.tile([128, TCH], BF16, tag="bure")
                buim = bp.tile([128, TCH], BF16, tag="buim")
                nc.scalar.copy(bure[:], pre[:])
                nc.scalar.copy(buim[:], pim[:])
                t1 = rp.tile([128, TCH], BF16, tag="t1")
                t2 = rp.tile([128, TCH], BF16, tag="t2")
                t3 = rp.tile([128, TCH], BF16, tag="t3")
                t4 = rp.tile([128, TCH], BF16, tag="t4")
                wre = wp.tile([128, TCH], BF16, tag="wre")
                wim = wp.tile([128, TCH], BF16, tag="wim")
                gre = gp.tile([128, TCH], BF16, tag="gre")
                gim = gp.tile([128, TCH], BF16, tag="gim")
                hre = hp.tile([128, TCH], BF16, tag="hre")
                him = hp.tile([128, TCH], BF16, tag="him")
                hist.append((c, hre, him))
                if halves is None:
                    halves = [slice(0, TCH)]
                for hi, hs in enumerate(halves):
                    cs_h, sn_h = csl[:, hs], snl[:, hs]
                    t1_h, t2_h = t1[:, hs], t2[:, hs]
                    # rotate into the r-frame: w = e^{-i theta t} * Bu
                    nc.vector.tensor_tensor(t1_h, cs_h, bure[:, hs], op=MUL)
                    nc.vector.tensor_tensor(t2_h, sn_h, buim[:, hs], op=MUL)
                    nc.vector.tensor_tensor(wre[:, hs], t1_h, t2_h, op=ADD)
                    nc.vector.tensor_tensor(t1_h, cs_h, buim[:, hs], op=MUL)
                    nc.vector.tensor_tensor(t2_h, sn_h, bure[:, hs], op=MUL)
                    nc.vector.tensor_tensor(wim[:, hs], t1_h, t2_h, op=SUB)
                    # the two real scans (f32 coeff + internal state)
                    if hi == 0:
                        init_re = 0.0 if c == 0 else prev_gre[:, TCH - 1:TCH]
                        init_im = 0.0 if c == 0 else prev_gim[:, TCH - 1:TCH]
                    else:
                        init_re = gre[:, hs.start - 1:hs.start]
                        init_im = gim[:, hs.start - 1:hs.start]
                    nc.vector.tensor_tensor_scan(
                        gre[:, hs], rbc[:, hs], wre[:, hs], init_re, MUL, ADD)
                    nc.vector.tensor_tensor_scan(
                        gim[:, hs], rbc[:, hs], wim[:, hs], init_im, MUL, ADD)
                    # rotate back: h = e^{+i theta t} * g
                    gre_h, gim_h = gre[:, hs], gim[:, hs]
                    # all four products into dedicated temps first, so him
                    # lands one op after hre and mm2's im-matmuls never
                    # stall waiting for it
                    nc.vector.tensor_tensor(t1_h, cs_h, gre_h, op=MUL)
                    nc.vector.tensor_tensor(t2_h, sn_h, gim_h, op=MUL)
                    nc.vector.tensor_tensor(t3[:, hs], cs_h, gim_h, op=MUL)
                    nc.vector.tensor_tensor(t4[:, hs], sn_h, gre_h, op=MUL)
                    nc.vector.tensor_tensor(hre[:, hs], t1_h, t2_h, op=SUB)
                    nc.vector.tensor_tensor(him[:, hs], t3[:, hs],
                                            t4[:, hs], op=ADD)
                    if post_half is not None:
                        post_half(hi)
                prev_gre, prev_gim = gre, gim

            def emit_back_tt(c, hre, him, tt):
                """One 128-timestep tile of mm2 + its drain + store."""
                t0 = c * TCH
                last_c = c == NCHUNK - 1
                lre = hre[:, tt * 128:(tt + 1) * 128]
                lim = him[:, tt * 128:(tt + 1) * 128]
                yo = yp.tile([128, H], BF16, tag="yo")
                pos = []
                for _ in range(NHC):
                    po = ps2.tile([128, HCH], F32, tag="po")
                    pos.append(po)
                for hc in range(NHC):
                    nc.tensor.matmul(
                        pos[hc][:], lre, ctr[:, hc * HCH:(hc + 1) * HCH],
                        start=True, stop=False,
                    )
                for hc in range(NHC):
                    nc.tensor.matmul(
                        pos[hc][:], lim, cti[:, hc * HCH:(hc + 1) * HCH],
                        start=False, stop=True,
                    )
                SSP = 288
                for hc in range(NHC):
                    ysl = yo[:, hc * HCH:(hc + 1) * HCH]
                    nc.scalar.copy(ysl[:, 0:SSP], pos[hc][:, 0:SSP])
                    nc.vector.tensor_copy(ysl[:, SSP:HCH],
                                          pos[hc][:, SSP:HCH])
                r0 = t0 + tt * 128
                if last_c:
                    # finest store pieces over both HWDGE queues: short tail
                    nc.sync.dma_start(ypart[r0:r0 + 32, :], yo[0:32, :])
                    nc.scalar.dma_start(ypart[r0 + 32:r0 + 64, :],
                                        yo[32:64, :])
                    nc.sync.dma_start(ypart[r0 + 64:r0 + 96, :], yo[64:96, :])
                    nc.scalar.dma_start(ypart[r0 + 96:r0 + 128, :],
                                        yo[96:128, :])
                else:
                    e0 = nc.gpsimd
                    e1 = nc.sync if tt % 2 == 0 else nc.gpsimd
                    e0.dma_start(ypart[r0:r0 + 64, :], yo[0:64, :])
                    e1.dma_start(ypart[r0 + 64:r0 + 128, :], yo[64:128, :])

            def emit_back():
                """mm2 + output for the oldest pending chunk."""
                c, hre, him = hist.pop(0)
                for tt in range(TCH // 128):
                    emit_back_tt(c, hre, him, tt)

            def emit_last_half(c, ctx, hi):
                """One F=256 half-time piece of the last chunk: mm1 half +
                cast + vector chain.  Half 0 is emitted before back(6) so
                its vector work overlaps mm2(6) on the other engines."""
                nonlocal prev_gre, prev_gim
                HT = TCH // 2
                hs = slice(0, HT) if hi == 0 else slice(HT, TCH)
                xt, cs_t, pre, pim = (ctx["xt"], ctx["cs"], ctx["pre"],
                                      ctx["pim"])
                for a in range(KT):
                    xsl = xt[:, a * TCH + hs.start:a * TCH + hs.stop]
                    nc.tensor.matmul(
                        pre[:, hs], bre[:, a * NSH:(a + 1) * NSH], xsl,
                        start=(a == 0), stop=(a == KT - 1),
                    )
                    nc.tensor.matmul(
                        pim[:, hs], bim[:, a * NSH:(a + 1) * NSH], xsl,
                        start=(a == 0), stop=(a == KT - 1),
                    )
                csl = cs_t[:, 0:TCH]
                snl = cs_t[:, TCH:2 * TCH]
                bure, buim = ctx["bure"], ctx["buim"]
                t1, t2 = ctx["t1"], ctx["t2"]
                wre, wim = ctx["wre"], ctx["wim"]
                gre, gim = ctx["gre"], ctx["gim"]
                hre, him = ctx["hre"], ctx["him"]
                nc.scalar.copy(bure[:, hs], pre[:, hs])
                nc.scalar.copy(buim[:, hs], pim[:, hs])
                cs_h, sn_h = csl[:, hs], snl[:, hs]
                t1_h, t2_h = t1[:, hs], t2[:, hs]
                nc.vector.tensor_tensor(t1_h, cs_h, bure[:, hs], op=MUL)
                nc.vector.tensor_tensor(t2_h, sn_h, buim[:, hs], op=MUL)
                nc.vector.tensor_tensor(wre[:, hs], t1_h, t2_h, op=ADD)
                nc.vector.tensor_tensor(t1_h, cs_h, buim[:, hs], op=MUL)
                nc.vector.tensor_tensor(t2_h, sn_h, bure[:, hs], op=MUL)
                nc.vector.tensor_tensor(wim[:, hs], t1_h, t2_h, op=SUB)
                if hi == 0:
                    init_re = prev_gre[:, TCH - 1:TCH]
                    init_im = prev_gim[:, TCH - 1:TCH]
                else:
                    init_re = gre[:, hs.start - 1:hs.start]
                    init_im = gim[:, hs.start - 1:hs.start]
                nc.vector.tensor_tensor_scan(
                    gre[:, hs], rbc[:, hs], wre[:, hs], init_re, MUL, ADD)
                nc.vector.tensor_tensor_scan(
                    gim[:, hs], rbc[:, hs], wim[:, hs], init_im, MUL, ADD)
                gre_h, gim_h = gre[:, hs], gim[:, hs]
                t3, t4 = ctx["t3"], ctx["t4"]
                nc.vector.tensor_tensor(t1_h, cs_h, gre_h, op=MUL)
                nc.vector.tensor_tensor(t2_h, sn_h, gim_h, op=MUL)
                nc.vector.tensor_tensor(t3[:, hs], cs_h, gim_h, op=MUL)
                nc.vector.tensor_tensor(t4[:, hs], sn_h, gre_h, op=MUL)
                nc.vector.tensor_tensor(hre[:, hs], t1_h, t2_h, op=SUB)
                nc.vector.tensor_tensor(him[:, hs], t3[:, hs],
                                        t4[:, hs], op=ADD)
                if hi == 1:
                    prev_gre, prev_gim = gre, gim

            LAST = NCHUNK - 1
            XPRE = {0: xt0, 1: xt1, 2: xt2}
            for c in range(LAST):
                emit_front(c, xt_pre=XPRE.get(c))
                if c >= 1:
                    emit_back()
            # Last chunk: half 0 (mm1 F=256 + vector chain) is emitted
            # before back(6), so TensorE runs mm2(6) while VectorE rotates
            # half 0; then half 1, then mm2(7) tiles chase the halves.
            xtL = xp.tile([128, KT * TCH], FP8, tag="xt")
            xL0 = LAST * KT * TCH
            QW = KT * TCH // 4
            for q in range(4):
                nc.sync.dma_start(xtL[:, q * QW:(q + 1) * QW],
                                  xT[:, xL0 + q * QW:xL0 + (q + 1) * QW])
            csL = cp.tile([128, 2 * TCH], BF16, tag="cs")
            nc.gpsimd.dma_start(
                csL[:], csT[:, LAST * 2 * TCH:(LAST + 1) * 2 * TCH])
            ctx = {"xt": xtL, "cs": csL}
            for nm, pool, dt in (
                ("pre", ps1, F32), ("pim", ps1, F32),
                ("bure", bp, BF16), ("buim", bp, BF16),
                ("t1", rp, BF16), ("t2", rp, BF16),
                ("t3", rp, BF16), ("t4", rp, BF16),
                ("wre", wp, BF16), ("wim", wp, BF16),
                ("gre", gp, BF16), ("gim", gp, BF16),
                ("hre", hp, BF16), ("him", hp, BF16),
            ):
                ctx[nm] = pool.tile([128, TCH], dt, tag=nm, name=f"L_{nm}")
            emit_last_half(LAST, ctx, 0)
            emit_back()  # chunk 6
            emit_last_half(LAST, ctx, 1)
            for tt in range(TCH // 128):
                emit_back_tt(LAST, ctx["hre"], ctx["him"], tt)

    nc.compile()
    return nc


def _arrange_bn(bn_slice):
    import ml_dtypes
    # bn_slice [NSH, H] (float64) -> [128, KT*NSH] with
    # out[p, a*NSH + n] = bn_slice[n, a*128 + p]
    bnT = bn_slice.T.astype(ml_dtypes.bfloat16)  # [H, NSH]
    return np.ascontiguousarray(
        bnT.reshape(KT, 128, NSH).transpose(1, 0, 2)).reshape(128, -1)


def _host_prep(inputs, nu, theta, gamma_log, B_re, B_im, C_re, C_im, D):
    """Float64 host-side precompute; returns per-core input maps."""
    import ml_dtypes
    BF = ml_dtypes.bfloat16
    x = np.asarray(inputs, dtype=np.float32)
    th64 = np.exp(np.asarray(theta).astype(np.float64))
    r64 = np.exp(-np.exp(np.asarray(nu).astype(np.float64)))
    gamma = np.exp(np.asarray(gamma_log).astype(np.float64))
    Bn_re = np.asarray(B_re).astype(np.float64) * gamma[:, None]
    Bn_im = np.asarray(B_im).astype(np.float64) * gamma[:, None]
    t_idx = np.arange(T, dtype=np.float64)
    phase = th64[:, None] * t_idx[None, :]
    cos_all = np.cos(phase).astype(BF)  # [N, T]
    sin_all = np.sin(phase).astype(BF)
    # merged per-chunk layout: cs_all[n, c, 0|1, t] = cos|sin(th_n*(c*TCH+t))
    cs_all = np.stack(
        [cos_all.reshape(N, NCHUNK, TCH), sin_all.reshape(N, NCHUNK, TCH)],
        axis=2).reshape(N, NCHUNK * 2 * TCH)
    # pre-arrange x into the per-chunk SBUF layout:
    # xTa[p, c, a, t] = x[c*TCH + t, a*128 + p]
    E3 = ml_dtypes.float8_e3m4
    xTa = np.ascontiguousarray(
        x.reshape(NCHUNK, TCH, KT, 128).transpose(3, 0, 2, 1).astype(E3)
    ).reshape(128, -1)
    C_re = np.asarray(C_re, dtype=np.float32).astype(BF)
    C_im = np.asarray(C_im, dtype=np.float32).astype(BF)

    in_maps = []
    for c in range(NCORES):
        sl = slice(c * NSH, (c + 1) * NSH)
        in_maps.append({
            "xT": xTa,
            "bn_re": _arrange_bn(Bn_re[sl]),
            "bn_im": _arrange_bn(Bn_im[sl]),
            "ct_re": np.ascontiguousarray(C_re[:, sl].T),
            "ct_in": np.ascontiguousarray(-C_im[:, sl].T),
            "csT": np.ascontiguousarray(cs_all[sl]),
            "rvec": np.ascontiguousarray(r64[sl].astype(np.float32)[:, None]),
        })
    return in_maps


def kernel(inputs, nu, theta, gamma_log, B_re, B_im, C_re, C_im, D):
    global last_results
    from concourse.bass_utils import run_bass_kernel_spmd

    if "nc" not in _CACHE:
        _CACHE["nc"] = _build_program()
    nc = _CACHE["nc"]

    in_maps = _host_prep(
        inputs, nu, theta, gamma_log, B_re, B_im, C_re, C_im, D)

    trace = os.environ.get("LRU_TRACE") == "1"
    res = run_bass_kernel_spmd(
        nc, in_maps, core_ids=list(range(NCORES)), trace=trace)
    last_results = res

    y64 = np.zeros((T, H), np.float64)
    for r in res.results:
        y64 += r["ypart"].astype(np.float64)
    y64 += (np.asarray(D).astype(np.float64)[None, :]
            * np.asarray(inputs).astype(np.float64))
    return y64.astype(np.float32)
